# revision 14
# baseline (speedup 1.0000x reference)
"""Trainium2 Bass kernel for nn_Diffuser (sparse_attention).

Math (per batch element b, fp32 reference):
    k = (ref @ Wk)                        -> (R, N, H, HD), kept transposed per head
    3x diffusion steps:
        qv = x @ Wqv ; q, v per head      (q scaled by SCALE here)
        attn  = q @ k^T                   per (r, h)        [K=64 contraction]
        attn2 = attn @ attn^T             (symmetric)       [K=256]
        y     = attn2 @ v ; mean over r   (1/R folded into v)
        x     = LayerNorm(reshape(y) @ Wproj + bproj)
    out = x

Sharding: pure data-parallel over B=8 across the 8 NeuronCores (weights and
ref replicated per core; zero collectives).

Layout strategy (per core): everything c-major ("transposed") so that all
matmuls contract over the partition dim; x stays transposed across steps,
with one transpose at load and one at store.  f32r (TF32-like) matmuls for
all O(N^2..N^3) work; fp32 accumulation in PSUM.
"""

import numpy as np
from contextlib import ExitStack

import concourse.bass as bass
import concourse.tile as tile
from concourse.tile_rust import add_dep_helper
from concourse import bacc, mybir
from concourse.bass_utils import run_bass_kernel_spmd
from concourse.masks import make_identity

F32 = mybir.dt.float32
F32R = mybir.dt.float32r
AF = mybir.ActivationFunctionType

D = 768
H = 12
HD = 64
R = 10
N = 256
STEPS = 3
NB = 8
CC = D // 128  # 6 column chunks of 128
SCALE = HD ** -0.5
EPS = 1e-5


def _emit(nc, tc, ctx, t_x, t_ref, t_Wqv, t_Wk, t_Wproj, t_bproj, t_gamma, t_beta, t_out):
    const = ctx.enter_context(tc.tile_pool(name="const", bufs=1))
    persist = ctx.enter_context(tc.tile_pool(name="persist", bufs=1))

    ident = const.tile([128, 128], F32)
    make_identity(nc, ident)

    ones_f = const.tile([128, 128], F32)
    nc.vector.memset(ones_f, 1.0)
    eps_sb = const.tile([128, 1], F32)
    nc.vector.memset(eps_sb, EPS)
    ones128 = const.tile([128, 128], F32R)
    nc.scalar.copy(ones128[:], ones_f[:])
    zsrc = const.tile([128, N], F32)
    nc.vector.memset(zsrc, 0.0)

    # per-chunk bias/scale vectors: (128, CC), column cc = chunk cc
    gamma_sb = const.tile([128, CC], F32)
    beta_sb = const.tile([128, CC], F32)
    bproj_sb = const.tile([128, CC], F32)
    nc.sync.dma_start(out=gamma_sb, in_=t_gamma.ap().rearrange("(c p) -> p c", p=128))
    nc.sync.dma_start(out=beta_sb, in_=t_beta.ap().rearrange("(c p) -> p c", p=128))
    nc.sync.dma_start(out=bproj_sb, in_=t_bproj.ap().rearrange("(c p) -> p c", p=128))

    # ---- resident weights (converted to f32r via compute copies) ----
    Wproj_r = []
    with tc.tile_pool(name="wload", bufs=3) as wload:
        for cc in range(CC):
            w = wload.tile([128, D], F32, tag="wproj")
            nc.sync.dma_start(out=w, in_=t_Wproj.ap()[cc * 128:(cc + 1) * 128, :])
            wr = persist.tile([128, D], F32R, tag=f"wprojr{cc}")
            nc.vector.tensor_copy(wr[:], w[:])
            Wproj_r.append(wr)

    # ---- x -> xT (c-major), f32r ----
    xT = [persist.tile([128, N], F32R, tag=f"xT{cc}", name=f"xT{cc}") for cc in range(CC)]
    with tc.tile_pool(name="xload", bufs=2) as xload, \
         tc.tile_pool(name="tps", bufs=3, space="PSUM") as tps:
        for nch in range(2):
            xn = xload.tile([128, D], F32, tag="xn")
            nc.sync.dma_start(out=xn, in_=t_x.ap()[nch * 128:(nch + 1) * 128, :])
            for cc in range(CC):
                pt = tps.tile([128, 128], F32, tag="pt")
                nc.tensor.transpose(pt[:], xn[:, cc * 128:(cc + 1) * 128], ident[:])
                nc.vector.tensor_copy(xT[cc][:, nch * 128:(nch + 1) * 128], pt[:])

    # ---- kT: k^T per (r, head-pair chunk), f32r, SBUF resident ----
    # kT[r][jc] is (128, 256): rows 0-63 head 2jc (d), 64-127 head 2jc+1
    kT = [[persist.tile([128, N], F32R, tag=f"kT{r}_{jc}", name=f"kT{r}_{jc}") for jc in range(CC)]
          for r in range(R)]
    with tc.tile_pool(name="wkload", bufs=3) as wkload, \
         tc.tile_pool(name="wkr", bufs=1) as wkrp, \
         tc.tile_pool(name="refload", bufs=3) as refload, \
         tc.tile_pool(name="reft", bufs=2) as reftp, \
         tc.tile_pool(name="tps2", bufs=3, space="PSUM") as tps2, \
         tc.tile_pool(name="kps", bufs=3, space="PSUM") as kps:
        Wk_r = []
        for cc in range(CC):
            w = wkload.tile([128, D], F32, tag="wkl")
            nc.sync.dma_start(out=w, in_=t_Wk.ap()[cc * 128:(cc + 1) * 128, :])
            wr = wkrp.tile([128, D], F32R, tag=f"wkr{cc}")
            nc.vector.tensor_copy(wr[:], w[:])
            Wk_r.append(wr)
        for r in range(R):
            refT = [reftp.tile([128, N], F32R, tag=f"refT{cc}", name=f"refT{r}_{cc}") for cc in range(CC)]
            for nch in range(2):
                rn = refload.tile([128, D], F32, tag="rn")
                nc.sync.dma_start(
                    out=rn, in_=t_ref.ap()[r * N + nch * 128: r * N + (nch + 1) * 128, :])
                for cc in range(CC):
                    pt = tps2.tile([128, 128], F32, tag="pt2")
                    nc.tensor.transpose(pt[:], rn[:, cc * 128:(cc + 1) * 128], ident[:])
                    nc.vector.tensor_copy(refT[cc][:, nch * 128:(nch + 1) * 128], pt[:])
            for jc in range(CC):
                pk = kps.tile([128, N], F32, tag="pk")
                for kc in range(CC):
                    nc.tensor.matmul(
                        pk[:], Wk_r[kc][:, jc * 128:(jc + 1) * 128], refT[kc][:],
                        start=(kc == 0), stop=(kc == CC - 1))
                nc.vector.tensor_copy(kT[r][jc][:], pk[:])

    # ---- persistent step tensors ----
    # qTz[h]: q^T for head h on its parity's partition rows, ZEROS on the
    # other 64 rows (lets B contract K=128 with full tiles — f32r needs them).
    qTz = [persist.tile([128, N], F32R, tag=f"qTz{h}", name=f"qTz{h}") for h in range(H)]
    # v_pad[pc][:, h*128:(h+1)*128]: v for head h in its parity's 64 columns,
    # ZEROS in the other 64 (lets D write full 128-row outputs; the zero rows
    # contribute +0 to the sibling head's z region).
    v_pad = [persist.tile([128, H * 128], F32R, tag=f"vp{p}", name=f"vp{p}")
             for p in range(2)]
    for h in range(H):
        par = h % 2
        nc.scalar.activation(qTz[h][(1 - par) * 64:(2 - par) * 64, :], zsrc[0:64, :],
                             AF.Copy, scale=0.0)
        for pc in range(2):
            nc.scalar.activation(
                v_pad[pc][:, h * 128 + (1 - par) * 64: h * 128 + (2 - par) * 64],
                zsrc[:, 0:64], AF.Copy, scale=0.0)
    m_sb = persist.tile([128, H * N], F32, tag="m_sb")
    zT = [persist.tile([128, N], F32R, tag=f"zT{cc}", name=f"zT{cc}") for cc in range(CC)]
    xp_sb = [persist.tile([128, N], F32R, tag=f"xp{cc}", name=f"xp{cc}") for cc in range(CC)]
    sq_sb = [persist.tile([128, N], F32R, tag=f"sq{cc}", name=f"sq{cc}") for cc in range(CC)]

    for step in range(STEPS):
        # ======== A: qv^T = Wqv^T @ x^T  (Wqv streamed from DRAM) ========
        with tc.tile_pool(name=f"qvps{step}", bufs=1, space="PSUM") as qvps, \
             tc.tile_pool(name=f"vtps{step}", bufs=2, space="PSUM") as vtps, \
             tc.tile_pool(name=f"wqv{step}", bufs=3) as wqvp, \
             tc.tile_pool(name=f"vtmp{step}", bufs=2) as vtmp:
            for half in range(2):  # half 0: q^T chunks; half 1: v^T chunks
                pqv = [qvps.tile([128, N], F32, tag=f"pqv{j}", name=f"pqv{step}_{half}_{j}") for j in range(CC)]
                for kc in range(CC):
                    w = wqvp.tile([128, D], F32, tag="wqvl")
                    nc.sync.dma_start(
                        out=w,
                        in_=t_Wqv.ap()[kc * 128:(kc + 1) * 128, half * D:(half + 1) * D])
                    wr = wqvp.tile([128, D], F32R, tag="wqvr")
                    nc.scalar.copy(wr[:], w[:])
                    for j in range(CC):
                        nc.tensor.matmul(
                            pqv[j][:], wr[:, j * 128:(j + 1) * 128], xT[kc][:],
                            start=(kc == 0), stop=(kc == CC - 1))
                for j in range(CC):
                    if half == 0:
                        # q^T chunk -> per-head zero-padded tiles, scaled
                        nc.scalar.activation(qTz[2 * j][0:64, :], pqv[j][0:64, :],
                                             AF.Copy, scale=SCALE)
                        nc.scalar.activation(qTz[2 * j + 1][64:128, :], pqv[j][64:128, :],
                                             AF.Copy, scale=SCALE)
                    else:
                        # v^T chunk (head-pair hp = j), scaled by 1/R;
                        # transpose, then scatter halves into v_pad slots
                        vt = vtmp.tile([128, N], F32, tag="vt")
                        nc.scalar.activation(vt[:], pqv[j][:], AF.Copy, scale=1.0 / R)
                        for nch in range(2):
                            pt = vtps.tile([128, 128], F32, tag="vpt")
                            nc.tensor.transpose(pt[:], vt[:, nch * 128:(nch + 1) * 128], ident[:])
                            nc.vector.tensor_copy(
                                v_pad[nch][:, (2 * j) * 128 + 0:(2 * j) * 128 + 64],
                                pt[:, 0:64])
                            nc.vector.tensor_copy(
                                v_pad[nch][:, (2 * j + 1) * 128 + 64:(2 * j + 1) * 128 + 128],
                                pt[:, 64:128])

        # ======== B/C/D: attention r-loop ========
        with tc.tile_pool(name=f"zps{step}", bufs=1, space="PSUM") as zps, \
             tc.tile_pool(name=f"bps{step}", bufs=2, space="PSUM") as bps, \
             tc.tile_pool(name=f"cps{step}", bufs=2, space="PSUM") as cps, \
             tc.tile_pool(name=f"atsb{step}", bufs=3) as atsb, \
             tc.tile_pool(name=f"a2sb{step}", bufs=3) as a2sb:
            zpsum = zps.tile([128, H * 128], F32, tag="z")  # (128, 1536): 3 banks
            zbank_start = {}
            for r in range(R):
                for h in range(H):
                    pair, par = h // 2, h % 2
                    # ---- B: attn^T = kT-chunk (stationary) x qTz[h] ----
                    # kT rows of the sibling head hit qTz's zero rows -> +0.
                    pb = bps.tile([128, 2 * N], F32, tag="B", name=f"pb{step}_{r}_{h}")
                    for mc in range(2):
                        nc.tensor.matmul(
                            pb[:, mc * N:(mc + 1) * N],
                            kT[r][pair][:, mc * 128:(mc + 1) * 128],
                            qTz[h][:],
                            start=True, stop=True)
                    a = atsb.tile([128, 2 * N], F32R, tag="AT", name=f"at{step}_{r}_{h}")
                    nc.vector.tensor_copy(a[:], pb[:])
                    # ---- C: attn2 = attn^T.T @ attn^T (accumulate m-chunks) ----
                    pc2 = cps.tile([128, 2 * N], F32, tag="C", name=f"pc2{step}_{r}_{h}")
                    for nch in range(2):
                        for mc in range(2):
                            nc.tensor.matmul(
                                pc2[:, nch * N:(nch + 1) * N],
                                a[:, mc * N + nch * 128: mc * N + (nch + 1) * 128],
                                a[:, mc * N:(mc + 1) * N],
                                start=(mc == 0), stop=(mc == 1))
                    a2t = a2sb.tile([128, 2 * N], F32R, tag="A2", name=f"a2t{step}_{r}_{h}")
                    if par == 0:
                        nc.vector.tensor_copy(a2t[:], pc2[:])
                    else:
                        nc.scalar.copy(a2t[:], pc2[:])
                    # ---- D: z[pair] += v_pad[h] (stationary) x attn2 ----
                    # v_pad's zero columns write +0 into the sibling's rows.
                    for pchunk in range(2):
                        bank = pair // 2
                        is_start = (r == 0 and pchunk == 0 and par == 0
                                    and pair % 2 == 0)
                        mi = nc.tensor.matmul(
                            zpsum[:, pair * 256:(pair + 1) * 256],
                            v_pad[pchunk][:, h * 128:(h + 1) * 128],
                            a2t[:, pchunk * N:(pchunk + 1) * N],
                            start=is_start,
                            stop=(r == R - 1 and pchunk == 1 and par == 1
                                  and pair % 2 == 1),
                            skip_group_check=True)
                        if is_start:
                            zbank_start[bank] = mi.ins
                        elif r == 0 and pchunk == 0 and par == 0 and pair % 2 == 1:
                            add_dep_helper(
                                mi.ins, zbank_start[bank], sync=False,
                                reason="z region first-write after bank start")

            # ======== E: drain z + duplicate halves + strided regather ========
            # m_sb cols h*256+n; even h valid on rows 0-63, odd h on rows 64-127
            for h in range(H):
                par = h % 2
                nc.vector.tensor_copy(
                    m_sb[par * 64:(par + 1) * 64, h * N:(h + 1) * N],
                    zpsum[par * 64:(par + 1) * 64, (h // 2) * 256 + 0:(h // 2) * 256 + N])
            # duplicate: even-h cols to rows 64-127, odd-h cols to rows 0-63
            ev = m_sb[0:64, :].rearrange("p (h n) -> p h n", n=N)[:, 0::2, :]
            ev_d = m_sb[64:128, :].rearrange("p (h n) -> p h n", n=N)[:, 0::2, :]
            od = m_sb[64:128, :].rearrange("p (h n) -> p h n", n=N)[:, 1::2, :]
            od_d = m_sb[0:64, :].rearrange("p (h n) -> p h n", n=N)[:, 1::2, :]
            nc.sync.dma_start(out=ev_d, in_=ev)
            nc.sync.dma_start(out=od_d, in_=od)
            # gather: zT[cc][cH%2 half] <- m_sb[half, cH::12] (t = 12*n' + cH)
            for cc in range(CC):
                nc.vector.tensor_copy(zT[cc][0:64, :], m_sb[0:64, 2 * cc::12])
                nc.vector.tensor_copy(zT[cc][64:128, :], m_sb[64:128, 2 * cc + 1::12])

        # ======== F: xp^T = Wproj^T @ z^T  (+ bproj) ========
        with tc.tile_pool(name=f"fps{step}", bufs=2, space="PSUM") as fps, \
             tc.tile_pool(name=f"sps{step}", bufs=2, space="PSUM") as sps, \
             tc.tile_pool(name=f"ln{step}", bufs=1) as ln:
            for mc in range(CC):
                pxp = fps.tile([128, N], F32, tag="pxp")
                for kc in range(CC):
                    nc.tensor.matmul(
                        pxp[:], Wproj_r[kc][:, mc * 128:(mc + 1) * 128], zT[kc][:],
                        start=(kc == 0), stop=(kc == CC - 1))
                nc.scalar.activation(
                    xp_sb[mc][:], pxp[:], AF.Identity, bias=bproj_sb[:, mc:mc + 1])

            # ======== G: LayerNorm over c (partition dim) ========
            # all-ones (128,128) stationary: every output row = column sum ->
            # stats arrive already broadcast across partitions.
            for mc in range(CC):
                nc.scalar.activation(sq_sb[mc][:], xp_sb[mc][:].bitcast(F32), AF.Square)
            psum_s = sps.tile([128, N], F32, tag="s", name=f"psum_s{step}")
            psum_q = sps.tile([128, N], F32, tag="q", name=f"psum_q{step}")
            for mc in range(CC):
                nc.tensor.matmul(psum_s[:], ones128[:], xp_sb[mc][:],
                                 start=(mc == 0), stop=(mc == CC - 1))
            for mc in range(CC):
                nc.tensor.matmul(psum_q[:], ones128[:], sq_sb[mc][:],
                                 start=(mc == 0), stop=(mc == CC - 1))
            mean_b = ln.tile([128, N], F32, tag="meanb")
            mean2_b = ln.tile([128, N], F32, tag="mean2b")
            var_b = ln.tile([128, N], F32, tag="varb")
            rsig_b = ln.tile([128, N], F32, tag="rsigb")
            nc.scalar.activation(mean_b[:], psum_s[:], AF.Copy, scale=1.0 / D)
            nc.vector.tensor_mul(mean2_b[:], mean_b[:], mean_b[:])
            nc.vector.scalar_tensor_tensor(
                out=var_b[:], in0=psum_q[:], scalar=1.0 / D, in1=mean2_b[:],
                op0=mybir.AluOpType.mult, op1=mybir.AluOpType.subtract)
            nc.scalar.activation(var_b[:], var_b[:], AF.Sqrt, bias=eps_sb[:])
            nc.vector.reciprocal(rsig_b[:], var_b[:])
            tmp = ln.tile([128, N], F32, tag="lntmp")
            for mc in range(CC):
                nc.vector.tensor_sub(tmp[:], xp_sb[mc][:].bitcast(F32), mean_b[:])
                nc.vector.scalar_tensor_tensor(
                    out=tmp[:], in0=tmp[:], scalar=gamma_sb[:, mc:mc + 1], in1=rsig_b[:],
                    op0=mybir.AluOpType.mult, op1=mybir.AluOpType.mult)
                nc.vector.tensor_scalar_add(
                    out=xT[mc][:], in0=tmp[:], scalar1=beta_sb[:, mc:mc + 1])

    # ======== epilogue: transpose x^T -> x, store ========
    with tc.tile_pool(name="eps", bufs=3, space="PSUM") as eps_pool, \
         tc.tile_pool(name="osb", bufs=1) as osb:
        out_nat = [osb.tile([128, D], F32, tag=f"on{nch}", name=f"on{nch}") for nch in range(2)]
        for cc in range(CC):
            for nch in range(2):
                pt = eps_pool.tile([128, 128], F32, tag="ept")
                nc.tensor.transpose(
                    pt[:], xT[cc][:, nch * 128:(nch + 1) * 128].bitcast(F32), ident[:])
                nc.vector.tensor_copy(out_nat[nch][:, cc * 128:(cc + 1) * 128], pt[:])
        for nch in range(2):
            nc.sync.dma_start(out=t_out.ap()[nch * 128:(nch + 1) * 128, :], in_=out_nat[nch][:])


def build():
    nc = bacc.Bacc("TRN2", target_bir_lowering=False, debug=False, num_devices=NB)
    t_x = nc.declare_dram_parameter("x", [N, D], F32, isOutput=False)
    t_ref = nc.declare_dram_parameter("ref", [R * N, D], F32, isOutput=False)
    t_Wqv = nc.declare_dram_parameter("Wqv", [D, 2 * D], F32, isOutput=False)
    t_Wk = nc.declare_dram_parameter("Wk", [D, D], F32, isOutput=False)
    t_Wproj = nc.declare_dram_parameter("Wproj", [D, D], F32, isOutput=False)
    t_bproj = nc.declare_dram_parameter("bproj", [D], F32, isOutput=False)
    t_gamma = nc.declare_dram_parameter("gamma", [D], F32, isOutput=False)
    t_beta = nc.declare_dram_parameter("beta", [D], F32, isOutput=False)
    t_out = nc.declare_dram_parameter("out", [N, D], F32, isOutput=True)
    with tile.TileContext(nc) as tc:
        with ExitStack() as ctx:
            _emit(nc, tc, ctx, t_x, t_ref, t_Wqv, t_Wk, t_Wproj, t_bproj,
                  t_gamma, t_beta, t_out)
    nc.compile()
    return nc


_CACHE = {}
last_results = None


def kernel(x, ref, Wqv, Wk, Wproj, bproj, gamma, beta):
    global last_results
    if "nc" not in _CACHE:
        _CACHE["nc"] = build()
    nc = _CACHE["nc"]

    def f(a):
        return np.ascontiguousarray(np.asarray(a), dtype=np.float32)

    x = f(x)
    common = dict(ref=f(ref).reshape(R * N, D), Wqv=f(Wqv), Wk=f(Wk),
                  Wproj=f(Wproj), bproj=f(bproj), gamma=f(gamma), beta=f(beta))
    in_maps = [dict(x=x[b], **common) for b in range(NB)]
    res = run_bass_kernel_spmd(nc, in_maps, list(range(NB)))
    last_results = res
    return np.stack([res.results[b]["out"] for b in range(NB)]).astype(np.float32)


# revision 18
# speedup vs baseline: 19.2419x; 19.2419x over previous
"""Trainium2 Bass kernel for nn_Diffuser (sparse_attention).

Key algebraic identity: the reference attention has NO softmax, so
    y_rh = (q k_rh^T s)(q k_rh^T s)^T v = s^2 * q (k_rh^T k_rh) (q^T v)
    mean_r y_rh = q @ Gbar_h @ (q_h^T v_h),
    Gbar_h = s^2/R * sum_r k_rh^T k_rh   (64x64, precomputed once).

Per step, per head: w = q^T v (64x64), P = Gbar w (64x64), z^T = P^T-as-lhsT
@ q^T.  The O(N^3) attention chain disappears entirely.

Sharding: pure data-parallel over B=8 across 8 NeuronCores (weights + ref
replicated, zero collectives).  All matmuls contract over the partition dim
(c-major layouts); f32r (TF32-like) full 128x128 tiles only — sub-tile shapes
are zero-padded (f32r rejects PE row/col tiling).
"""

import numpy as np
from contextlib import ExitStack

import concourse.bass as bass
import concourse.tile as tile
from concourse import bacc, mybir
from concourse.bass_utils import run_bass_kernel_spmd
from concourse.masks import make_identity
from concourse.tile_rust import add_dep_helper

F32 = mybir.dt.float32
F32R = mybir.dt.float32r
AF = mybir.ActivationFunctionType

D = 768
H = 12
HD = 64
R = 10
N = 256
STEPS = 3
NB = 8
CC = D // 128
SCALE = HD ** -0.5
EPS = 1e-5
GS = SCALE * SCALE / R  # folded into Gbar


def _emit(nc, tc, ctx, t_x, t_ref, t_Wqv, t_Wk, t_Wproj, t_bproj, t_gamma, t_beta, t_out,
          iters=1):
    const = ctx.enter_context(tc.tile_pool(name="const", bufs=1))
    persist = ctx.enter_context(tc.tile_pool(name="persist", bufs=1))

    ident = const.tile([128, 128], F32)
    make_identity(nc, ident)
    ones_f = const.tile([128, 128], F32)
    nc.vector.memset(ones_f, 1.0)
    eps_sb = const.tile([128, 1], F32)
    nc.vector.memset(eps_sb, EPS)
    ones128 = const.tile([128, 128], F32R)
    nc.scalar.copy(ones128[:], ones_f[:])
    zsrc = const.tile([128, N], F32)
    nc.vector.memset(zsrc, 0.0)

    gamma_sb = const.tile([128, CC], F32)
    beta_sb = const.tile([128, CC], F32)
    bproj_sb = const.tile([128, CC], F32)
    nc.sync.dma_start(out=gamma_sb, in_=t_gamma.ap().rearrange("(c p) -> p c", p=128))
    nc.sync.dma_start(out=beta_sb, in_=t_beta.ap().rearrange("(c p) -> p c", p=128))
    nc.sync.dma_start(out=bproj_sb, in_=t_bproj.ap().rearrange("(c p) -> p c", p=128))

    # ---- resident weights ----
    Wproj_r = []
    with tc.tile_pool(name="wload", bufs=3) as wload:
        for cc in range(CC):
            w = wload.tile([128, D], F32, tag="wproj")
            nc.sync.dma_start(out=w, in_=t_Wproj.ap()[cc * 128:(cc + 1) * 128, :])
            wr = persist.tile([128, D], F32R, tag=f"wprojr{cc}")
            nc.vector.tensor_copy(wr[:], w[:])
            Wproj_r.append(wr)

    xT = [persist.tile([128, N], F32R, tag=f"xT{cc}", name=f"xT{cc}") for cc in range(CC)]
    qT = [persist.tile([128, N], F32R, tag=f"qT{cc}", name=f"qT{cc}") for cc in range(CC)]
    v_pad = [persist.tile([128, H * 128], F32R, tag=f"vp{p}", name=f"vp{p}")
             for p in range(2)]
    qn = [persist.tile([128, 2 * 128], F32R, tag=f"qn{h}", name=f"qn{h}") for h in range(H)]
    G_sb = [persist.tile([128, 128], F32R, tag=f"G{h}", name=f"G{h}") for h in range(H)]
    Pz = [persist.tile([128, 128], F32R, tag=f"Pz{h}", name=f"Pz{h}") for h in range(H)]
    w_sb = [persist.tile([128, HD], F32R, tag=f"w{h}", name=f"w{h}") for h in range(H)]
    m_sb = persist.tile([128, H * N], F32, tag="m_sb")
    zT = [persist.tile([128, N], F32R, tag=f"zT{cc}", name=f"zT{cc}") for cc in range(CC)]
    xp_sb = [persist.tile([128, N], F32R, tag=f"xp{cc}", name=f"xp{cc}") for cc in range(CC)]
    sq_sb = [persist.tile([128, N], F32R, tag=f"sq{cc}", name=f"sq{cc}") for cc in range(CC)]

    # zero-fill pads once (via ACT so f32r consumers see a rounding producer)
    for h in range(H):
        par = h % 2
        for pc in range(2):
            nc.scalar.activation(
                v_pad[pc][:, h * 128 + (1 - par) * 64: h * 128 + (2 - par) * 64],
                zsrc[:, 0:64], AF.Copy, scale=0.0)
        for nch in range(2):
            nc.scalar.activation(qn[h][:, nch * 128 + 64: nch * 128 + 128],
                                 zsrc[:, 0:64], AF.Copy, scale=0.0)
        nc.scalar.activation(G_sb[h][:], zsrc[:, 0:128], AF.Copy, scale=0.0)
        nc.scalar.activation(Pz[h][:, (1 - par) * 64:(2 - par) * 64],
                             zsrc[:, 0:64], AF.Copy, scale=0.0)

    def one_pass(it):
        # ---- x -> xT (c-major) ----
        with tc.tile_pool(name=f"xload{it}", bufs=2) as xload, \
             tc.tile_pool(name=f"tps{it}", bufs=3, space="PSUM") as tps:
            for nch in range(2):
                xn = xload.tile([128, D], F32, tag="xn")
                nc.sync.dma_start(out=xn, in_=t_x.ap()[nch * 128:(nch + 1) * 128, :])
                for cc in range(CC):
                    pt = tps.tile([128, 128], F32, tag="pt")
                    nc.tensor.transpose(pt[:], xn[:, cc * 128:(cc + 1) * 128], ident[:])
                    nc.vector.tensor_copy(xT[cc][:, nch * 128:(nch + 1) * 128], pt[:])

        # ---- Gbar_h = s^2/R * sum_r k_rh^T k_rh  (64x64 per head) ----
        with tc.tile_pool(name=f"wkload{it}", bufs=3) as wkload, \
             tc.tile_pool(name=f"wkr{it}", bufs=1) as wkrp, \
             tc.tile_pool(name=f"refload{it}", bufs=3) as refload, \
             tc.tile_pool(name=f"reft{it}", bufs=2) as reftp, \
             tc.tile_pool(name=f"kn{it}", bufs=1) as knp, \
             tc.tile_pool(name=f"tps2{it}", bufs=3, space="PSUM") as tps2, \
             tc.tile_pool(name=f"kps{it}", bufs=3, space="PSUM") as kps, \
             tc.tile_pool(name=f"gps{it}", bufs=2, space="PSUM") as gps:
            Wk_r = []
            for cc in range(CC):
                w = wkload.tile([128, D], F32, tag="wkl")
                nc.sync.dma_start(out=w, in_=t_Wk.ap()[cc * 128:(cc + 1) * 128, :])
                wr = wkrp.tile([128, D], F32R, tag=f"wkr{cc}")
                nc.vector.tensor_copy(wr[:], w[:])
                Wk_r.append(wr)
            # k natural (m-major), 5 refs resident at a time; cols 768:832
            # zeroed (pad for the sliding head-pair lhsT of the Gram matmuls)
            RG = R // 2
            for rg in range(2):
                kn = [[knp.tile([128, D + HD], F32R, tag=f"kn{rr}_{mch}",
                                name=f"kn{it}_{rg}_{rr}_{mch}") for mch in range(2)]
                      for rr in range(RG)]
                for rr in range(RG):
                    r = rg * RG + rr
                    refT = [reftp.tile([128, N], F32R, tag=f"refT{cc}",
                                       name=f"refT{it}_{r}_{cc}") for cc in range(CC)]
                    for nch in range(2):
                        rn = refload.tile([128, D], F32, tag="rn")
                        nc.sync.dma_start(
                            out=rn, in_=t_ref.ap()[r * N + nch * 128: r * N + (nch + 1) * 128, :])
                        for cc in range(CC):
                            pt = tps2.tile([128, 128], F32, tag="pt2")
                            nc.tensor.transpose(pt[:], rn[:, cc * 128:(cc + 1) * 128], ident[:])
                            if cc % 2 == 0:
                                nc.vector.tensor_copy(refT[cc][:, nch * 128:(nch + 1) * 128], pt[:])
                            else:
                                nc.scalar.copy(refT[cc][:, nch * 128:(nch + 1) * 128], pt[:])
                    for mch in range(2):
                        nc.scalar.activation(kn[rr][mch][:, D:D + HD], zsrc[:, 0:64],
                                             AF.Copy, scale=0.0)
                        for jh in range(2):  # Nf chunks of 384
                            pk = kps.tile([128, 384], F32, tag="pk")
                            for kc in range(CC):
                                nc.tensor.matmul(
                                    pk[:], refT[kc][:, mch * 128:(mch + 1) * 128],
                                    Wk_r[kc][:, jh * 384:(jh + 1) * 384],
                                    start=(kc == 0), stop=(kc == CC - 1))
                            if jh == 0:
                                nc.vector.tensor_copy(kn[rr][mch][:, 0:384], pk[:])
                            else:
                                nc.scalar.copy(kn[rr][mch][:, 384:768], pk[:])
                for h in range(H):
                    par = h % 2
                    pg = gps.tile([128, HD], F32, tag="pg", name=f"pg{it}_{rg}_{h}")
                    for rr in range(RG):
                        for mch in range(2):
                            nc.tensor.matmul(
                                pg[:], kn[rr][mch][:, h * 64: h * 64 + 128],
                                kn[rr][mch][:, h * 64: h * 64 + 64],
                                start=(rr == 0 and mch == 0),
                                stop=(rr == RG - 1 and mch == 1))
                    # rows 0-63 = Gram block, scaled into G_sb col-parity slot
                    if rg == 0:
                        nc.scalar.activation(G_sb[h][0:64, par * 64:(par + 1) * 64],
                                             pg[0:64, :], AF.Copy, scale=GS)
                    else:
                        nc.vector.scalar_tensor_tensor(
                            out=G_sb[h][0:64, par * 64:(par + 1) * 64],
                            in0=pg[0:64, :], scalar=GS,
                            in1=G_sb[h][0:64, par * 64:(par + 1) * 64].bitcast(F32),
                            op0=mybir.AluOpType.mult, op1=mybir.AluOpType.add)

        for step in range(STEPS):
            # ---- A: qv^T = Wqv^T @ x^T (streamed weights) ----
            with tc.tile_pool(name=f"qvps{it}_{step}", bufs=1, space="PSUM") as qvps, \
                 tc.tile_pool(name=f"vtps{it}_{step}", bufs=2, space="PSUM") as vtps, \
                 tc.tile_pool(name=f"wqv{it}_{step}", bufs=3) as wqvp, \
                 tc.tile_pool(name=f"vtmp{it}_{step}", bufs=2) as vtmp:
                for half in range(2):
                    pqv = [qvps.tile([128, N], F32, tag=f"pqv{j}",
                                     name=f"pqv{it}_{step}_{half}_{j}") for j in range(CC)]
                    for kc in range(CC):
                        w = wqvp.tile([128, D], F32, tag="wqvl")
                        nc.sync.dma_start(
                            out=w,
                            in_=t_Wqv.ap()[kc * 128:(kc + 1) * 128, half * D:(half + 1) * D])
                        wr = wqvp.tile([128, D], F32R, tag="wqvr")
                        nc.scalar.copy(wr[:], w[:])
                        for j in range(CC):
                            nc.tensor.matmul(
                                pqv[j][:], wr[:, j * 128:(j + 1) * 128], xT[kc][:],
                                start=(kc == 0), stop=(kc == CC - 1))
                    for j in range(CC):
                        if half == 0:
                            nc.scalar.copy(qT[j][:], pqv[j][:])
                        else:
                            vt = vtmp.tile([128, N], F32, tag="vt")
                            nc.scalar.copy(vt[:], pqv[j][:])
                            for nch in range(2):
                                pt = vtps.tile([128, 128], F32, tag="vpt")
                                nc.tensor.transpose(pt[:], vt[:, nch * 128:(nch + 1) * 128],
                                                    ident[:])
                                nc.vector.tensor_copy(
                                    v_pad[nch][:, (2 * j) * 128 + 0:(2 * j) * 128 + 64],
                                    pt[:, 0:64])
                                nc.vector.tensor_copy(
                                    v_pad[nch][:, (2 * j + 1) * 128 + 64:(2 * j + 1) * 128 + 128],
                                    pt[:, 64:128])

            # ---- attention: q-nat transposes, w, P, z ----
            with tc.tile_pool(name=f"zps{it}_{step}", bufs=1, space="PSUM") as zps, \
                 tc.tile_pool(name=f"sps2{it}_{step}", bufs=2, space="PSUM") as sps2, \
                 tc.tile_pool(name=f"wps{it}_{step}", bufs=1, space="PSUM") as wps:
                zpsum = zps.tile([128, H * 128], F32, tag="z",
                                 name=f"zpsum{it}_{step}")  # 3 banks
                zbank_start = {}
                # q natural: one full-tile transpose per (pair, nch); the two
                # heads come out side by side in the free dim
                for pair in range(CC):
                    for nch in range(2):
                        pt = sps2.tile([128, 128], F32, tag="qnt")
                        nc.tensor.transpose(
                            pt[:], qT[pair][:, nch * 128:(nch + 1) * 128].bitcast(F32),
                            ident[:])
                        nc.vector.tensor_copy(
                            qn[2 * pair][:, nch * 128: nch * 128 + 64], pt[:, 0:64])
                        nc.vector.tensor_copy(
                            qn[2 * pair + 1][:, nch * 128: nch * 128 + 64], pt[:, 64:128])
                for h in range(H):
                    pair, par = h // 2, h % 2
                    # w = q^T v  (64x64, rows 0-63; rows 64-127 genuine zeros)
                    pw = wps.tile([128, HD], F32, tag="w", name=f"pw{it}_{step}_{h}")
                    for nch in range(2):
                        nc.tensor.matmul(
                            pw[:], qn[h][:, nch * 128:(nch + 1) * 128],
                            v_pad[nch][:, h * 128 + par * 64: h * 128 + (par + 1) * 64],
                            start=(nch == 0), stop=(nch == 1))
                    nc.vector.tensor_copy(w_sb[h][:], pw[:])
                    # P = Gbar w  (lands on rows par*64.. via G_sb col parity)
                    pp = wps.tile([128, HD], F32, tag="p", name=f"pp{it}_{step}_{h}")
                    nc.tensor.matmul(pp[:], G_sb[h][:], w_sb[h][:],
                                     start=True, stop=True)
                    nc.scalar.copy(Pz[h][:, par * 64:(par + 1) * 64], pp[:])
                    # z^T[pair] += P^T-as-lhsT @ q^T (par-packed output rows)
                    bank = pair // 2
                    is_start = (par == 0 and pair % 2 == 0)
                    mi = nc.tensor.matmul(
                        zpsum[:, pair * 256:(pair + 1) * 256],
                        Pz[h][:], qT[pair][:],
                        start=is_start,
                        stop=(par == 1 and pair % 2 == 1),
                        skip_group_check=True)
                    if is_start:
                        zbank_start[bank] = mi.ins
                    elif par == 0 and pair % 2 == 1:
                        add_dep_helper(mi.ins, zbank_start[bank], sync=False,
                                       reason="z region first-write after bank start")

                # ---- E: drain z + duplicate halves + strided regather ----
                for h in range(H):
                    par = h % 2
                    nc.vector.tensor_copy(
                        m_sb[par * 64:(par + 1) * 64, h * N:(h + 1) * N],
                        zpsum[par * 64:(par + 1) * 64, (h // 2) * 256:(h // 2) * 256 + N])
                ev = m_sb[0:64, :].rearrange("p (h n) -> p h n", n=N)[:, 0::2, :]
                ev_d = m_sb[64:128, :].rearrange("p (h n) -> p h n", n=N)[:, 0::2, :]
                od = m_sb[64:128, :].rearrange("p (h n) -> p h n", n=N)[:, 1::2, :]
                od_d = m_sb[0:64, :].rearrange("p (h n) -> p h n", n=N)[:, 1::2, :]
                nc.sync.dma_start(out=ev_d, in_=ev)
                nc.sync.dma_start(out=od_d, in_=od)
                for cc in range(CC):
                    nc.vector.tensor_copy(zT[cc][0:64, :], m_sb[0:64, 2 * cc::12])
                    nc.vector.tensor_copy(zT[cc][64:128, :], m_sb[64:128, 2 * cc + 1::12])

            # ---- F: xp^T = Wproj^T @ z^T (+bproj);  G: LayerNorm over c ----
            with tc.tile_pool(name=f"fps{it}_{step}", bufs=2, space="PSUM") as fps, \
                 tc.tile_pool(name=f"sps{it}_{step}", bufs=2, space="PSUM") as sps, \
                 tc.tile_pool(name=f"ln{it}_{step}", bufs=1) as ln:
                for mc in range(CC):
                    pxp = fps.tile([128, N], F32, tag="pxp")
                    for kc in range(CC):
                        nc.tensor.matmul(
                            pxp[:], Wproj_r[kc][:, mc * 128:(mc + 1) * 128], zT[kc][:],
                            start=(kc == 0), stop=(kc == CC - 1))
                    nc.scalar.activation(
                        xp_sb[mc][:], pxp[:], AF.Identity, bias=bproj_sb[:, mc:mc + 1])
                for mc in range(CC):
                    nc.scalar.activation(sq_sb[mc][:], xp_sb[mc][:].bitcast(F32), AF.Square)
                psum_s = sps.tile([128, N], F32, tag="s", name=f"psum_s{it}_{step}")
                psum_q = sps.tile([128, N], F32, tag="q", name=f"psum_q{it}_{step}")
                for mc in range(CC):
                    nc.tensor.matmul(psum_s[:], ones128[:], xp_sb[mc][:],
                                     start=(mc == 0), stop=(mc == CC - 1))
                for mc in range(CC):
                    nc.tensor.matmul(psum_q[:], ones128[:], sq_sb[mc][:],
                                     start=(mc == 0), stop=(mc == CC - 1))
                mean_b = ln.tile([128, N], F32, tag="meanb")
                mean2_b = ln.tile([128, N], F32, tag="mean2b")
                var_b = ln.tile([128, N], F32, tag="varb")
                rsig_b = ln.tile([128, N], F32, tag="rsigb")
                nc.scalar.activation(mean_b[:], psum_s[:], AF.Copy, scale=1.0 / D)
                nc.vector.tensor_mul(mean2_b[:], mean_b[:], mean_b[:])
                nc.vector.scalar_tensor_tensor(
                    out=var_b[:], in0=psum_q[:], scalar=1.0 / D, in1=mean2_b[:],
                    op0=mybir.AluOpType.mult, op1=mybir.AluOpType.subtract)
                nc.scalar.activation(var_b[:], var_b[:], AF.Sqrt, bias=eps_sb[:])
                nc.vector.reciprocal(rsig_b[:], var_b[:])
                tmp = ln.tile([128, N], F32, tag="lntmp")
                for mc in range(CC):
                    nc.vector.tensor_sub(tmp[:], xp_sb[mc][:].bitcast(F32), mean_b[:])
                    nc.vector.scalar_tensor_tensor(
                        out=tmp[:], in0=tmp[:], scalar=gamma_sb[:, mc:mc + 1],
                        in1=rsig_b[:],
                        op0=mybir.AluOpType.mult, op1=mybir.AluOpType.mult)
                    nc.vector.tensor_scalar_add(
                        out=xT[mc][:], in0=tmp[:], scalar1=beta_sb[:, mc:mc + 1])

        # ---- epilogue: transpose x^T -> x, store ----
        with tc.tile_pool(name=f"eps{it}", bufs=3, space="PSUM") as eps_pool, \
             tc.tile_pool(name=f"osb{it}", bufs=1) as osb:
            out_nat = [osb.tile([128, D], F32, tag=f"on{it}_{nch}",
                                name=f"on{it}_{nch}") for nch in range(2)]
            for cc in range(CC):
                for nch in range(2):
                    pt = eps_pool.tile([128, 128], F32, tag="ept")
                    nc.tensor.transpose(
                        pt[:], xT[cc][:, nch * 128:(nch + 1) * 128].bitcast(F32), ident[:])
                    nc.vector.tensor_copy(out_nat[nch][:, cc * 128:(cc + 1) * 128], pt[:])
            for nch in range(2):
                nc.sync.dma_start(out=t_out.ap()[nch * 128:(nch + 1) * 128, :],
                                  in_=out_nat[nch][:])

    if iters == 1:
        one_pass(0)
    else:
        with tc.For_i(0, iters, 1):
            one_pass(0)


def build(iters=1):
    nc = bacc.Bacc("TRN2", target_bir_lowering=False, debug=False, num_devices=NB)
    t_x = nc.declare_dram_parameter("x", [N, D], F32, isOutput=False)
    t_ref = nc.declare_dram_parameter("ref", [R * N, D], F32, isOutput=False)
    t_Wqv = nc.declare_dram_parameter("Wqv", [D, 2 * D], F32, isOutput=False)
    t_Wk = nc.declare_dram_parameter("Wk", [D, D], F32, isOutput=False)
    t_Wproj = nc.declare_dram_parameter("Wproj", [D, D], F32, isOutput=False)
    t_bproj = nc.declare_dram_parameter("bproj", [D], F32, isOutput=False)
    t_gamma = nc.declare_dram_parameter("gamma", [D], F32, isOutput=False)
    t_beta = nc.declare_dram_parameter("beta", [D], F32, isOutput=False)
    t_out = nc.declare_dram_parameter("out", [N, D], F32, isOutput=True)
    with tile.TileContext(nc) as tc:
        with ExitStack() as ctx:
            _emit(nc, tc, ctx, t_x, t_ref, t_Wqv, t_Wk, t_Wproj, t_bproj,
                  t_gamma, t_beta, t_out, iters=iters)
    nc.compile()
    return nc


_CACHE = {}
last_results = None


def kernel(x, ref, Wqv, Wk, Wproj, bproj, gamma, beta):
    global last_results
    if "nc" not in _CACHE:
        _CACHE["nc"] = build()
    nc = _CACHE["nc"]

    def f(a):
        return np.ascontiguousarray(np.asarray(a), dtype=np.float32)

    x = f(x)
    common = dict(ref=f(ref).reshape(R * N, D), Wqv=f(Wqv), Wk=f(Wk),
                  Wproj=f(Wproj), bproj=f(bproj), gamma=f(gamma), beta=f(beta))
    in_maps = [dict(x=x[b], **common) for b in range(NB)]
    res = run_bass_kernel_spmd(nc, in_maps, list(range(NB)))
    last_results = res
    return np.stack([res.results[b]["out"] for b in range(NB)]).astype(np.float32)
